# revision 1
# baseline (speedup 1.0000x reference)
"""Trainium2 Bass kernel for CostVolumePrompt (masked-softmax cost volume).

Computation per (b, h):
  vol[i, j] = dot(lfeat[b,:,h,i], rfeat[b,:,h,j]) / sqrt(C)      (W x W)
  prob      = softmax(vol, axis=j) * tril(W, W)                  (mask AFTER softmax)
  corresp_i = sum_j prob[i,j] * j
  conf_i    = max_j prob[i,j]
  disp_i    = max((i - corresp_i)/W, 0.1)
  out       = [fx*baseline/lfar / disp, conf]

Device strategy (8 cores, data-parallel over H): each core owns H/8 = 16 rows
for all 4 batches -> 64 (b,h) pairs; per pair, 4 row-tiles of (128 i x 512 j).
Engine balance per pair:
  PE  : 4 matmuls (f16 lhsT/rhs) -> vol tiles in PSUM
  ACT : exp(vol/sqrt(C)); tiles 0+1 batched in one inst (no accum),
        tiles 2,3 separate with accum_out -> their denominators
  GPS : denominators of tiles 0,1 (scalar_tensor_tensor sum-accum)
        and masked weighted sums s1 of tiles 0,1
  DVE : masked max (conf) for all tiles, s1 for tiles 2,3
"""

import math
import numpy as np
from contextlib import ExitStack

import concourse.bass as bass
import concourse.bacc as bacc
import concourse.tile as tile
from concourse import mybir
from concourse._compat import with_exitstack
from concourse.bass_utils import run_bass_kernel_spmd
from concourse.dve_ops import TENSOR_TENSOR_REDUCE, TENSOR_MASK_REDUCE

B, V, C, H, W = 4, 2, 128, 128, 512
NCORES = 8
HLOC = H // NCORES          # 16 h-rows per core
NT = HLOC * 4               # finals columns per batch (h*4 + mi) = 64
SCALE = 1.0 / math.sqrt(C)  # 1/sqrt(C) / TEMPERATURE
MIN_DISP = 0.1

F32 = mybir.dt.float32
F16 = mybir.dt.float16


@with_exitstack
def _body(ctx: ExitStack, tc: "tile.TileContext", io: dict):
    nc = tc.nc
    lfeat, rfeat = io["lfeat"], io["rfeat"]
    widx, maskend, ivec, scales = io["widx"], io["maskend"], io["ivec"], io["scales"]
    out_dc, out_cf = io["out_dc"], io["out_cf"]

    singles = ctx.enter_context(tc.tile_pool(name="singles", bufs=1))
    feats = ctx.enter_context(tc.tile_pool(name="feats", bufs=10))
    psumA = ctx.enter_context(tc.tile_pool(name="psumA", bufs=2, space="PSUM"))
    psumB = ctx.enter_context(tc.tile_pool(name="psumB", bufs=2, space="PSUM"))
    epool = ctx.enter_context(tc.tile_pool(name="epool", bufs=4))
    scr = ctx.enter_context(tc.tile_pool(name="scr", bufs=4))
    acc = ctx.enter_context(tc.tile_pool(name="acc", bufs=2))
    fin = ctx.enter_context(tc.tile_pool(name="fin", bufs=2))

    widx_sb = singles.tile([128, 4, W], F32)
    nc.sync.dma_start(out=widx_sb[:], in_=widx[:, :, :])
    maskend_sb = singles.tile([128, 4], F32)
    nc.sync.dma_start(out=maskend_sb[:], in_=maskend[:, :])
    ivec_sb = singles.tile([128, NT], F32)
    nc.sync.dma_start(out=ivec_sb[:], in_=ivec[:, :])
    sc_sb = singles.tile([128, B], F32)
    nc.sync.dma_start(out=sc_sb[:], in_=scales[:, :].to_broadcast((128, B)))
    ones_sb = singles.tile([128, W], F32)
    nc.vector.memset(ones_sb[:], 1.0)

    for b in range(B):
        s1c = acc.tile([128, NT], F32, tag="s1c")
        dnc = acc.tile([128, NT], F32, tag="dnc")
        c1c = acc.tile([128, NT], F32, tag="c1c")
        for h in range(HLOC):
            lf = feats.tile([128, W], F16, tag="lf")
            rf = feats.tile([128, W], F16, tag="rf")
            nc.sync.dma_start(out=lf[:], in_=lfeat[b, :, h, :])
            nc.sync.dma_start(out=rf[:], in_=rfeat[b, :, h, :])
            t0 = h * 4

            # ---- PE: 4 matmuls; tiles 0+1 into one 2-bank PSUM tile ----
            vol01 = psumA.tile([128, 2 * W], F32, tag="vol01")
            nc.tensor.matmul(vol01[:, 0:W], lf[:, 0:128], rf[:, :],
                             start=True, stop=True)
            nc.tensor.matmul(vol01[:, W:2 * W], lf[:, 128:256], rf[:, :],
                             start=True, stop=True)
            vol2 = psumB.tile([128, W], F32, tag="vol2")
            nc.tensor.matmul(vol2[:], lf[:, 256:384], rf[:, :],
                             start=True, stop=True)
            vol3 = psumB.tile([128, W], F32, tag="vol3")
            nc.tensor.matmul(vol3[:], lf[:, 384:512], rf[:, :],
                             start=True, stop=True)

            # ---- ACT: exp per tile with accum_out -> denominators ----
            e01 = epool.tile([128, 2 * W], F32, tag="e01")
            nc.scalar.activation(out=e01[:, 0:W], in_=vol01[:, 0:W],
                                 func=mybir.ActivationFunctionType.Exp,
                                 scale=SCALE, accum_out=dnc[:, t0:t0 + 1])
            nc.scalar.activation(out=e01[:, W:2 * W], in_=vol01[:, W:2 * W],
                                 func=mybir.ActivationFunctionType.Exp,
                                 scale=SCALE, accum_out=dnc[:, t0 + 1:t0 + 2])
            e2 = epool.tile([128, W], F32, tag="e2")
            nc.scalar.activation(out=e2[:], in_=vol2[:],
                                 func=mybir.ActivationFunctionType.Exp,
                                 scale=SCALE, accum_out=dnc[:, t0 + 2:t0 + 3])
            e3 = epool.tile([128, W], F32, tag="e3")
            nc.scalar.activation(out=e3[:], in_=vol3[:],
                                 func=mybir.ActivationFunctionType.Exp,
                                 scale=SCALE, accum_out=dnc[:, t0 + 3:t0 + 4])

            # ---- s1: all tiles on DVE (TTR) ----
            for mi in (0, 1):
                ext = (mi + 1) * 128
                esl = e01[:, 0:128] if mi == 0 else e01[:, W:W + 256]
                so = scr.tile([128, ext], F32, tag=f"so{mi}")
                nc.vector._custom_dve(
                    TENSOR_TENSOR_REDUCE, out=so[:], in0=esl,
                    in1=widx_sb[:, mi, 0:ext], s0=0.0, s1=1.0,
                    accum_out=s1c[:, t0 + mi:t0 + mi + 1])
            for mi in (2, 3):
                ext = (mi + 1) * 128
                esl = e2[:, 0:384] if mi == 2 else e3[:, 0:W]
                so = scr.tile([128, ext], F32, tag=f"so{mi}")
                nc.vector._custom_dve(
                    TENSOR_TENSOR_REDUCE, out=so[:], in0=esl,
                    in1=widx_sb[:, mi, 0:ext], s0=0.0, s1=1.0,
                    accum_out=s1c[:, t0 + mi:t0 + mi + 1])
            exts = (128, 256, 384, 512)
            esrc = {0: e01[:, 0:128], 1: e01[:, W:W + 256],
                    2: e2[:, 0:384], 3: e3[:, 0:W]}
            for mi in range(4):
                mo = scr.tile([128, exts[mi]], F32, tag=f"mo{mi}")
                nc.vector._custom_dve(
                    TENSOR_MASK_REDUCE, out=mo[:], in0=esrc[mi],
                    in1=maskend_sb[:, mi:mi + 1], s0=0.0, s1=0.0, imm2=1.0,
                    accum_out=c1c[:, t0 + mi:t0 + mi + 1])

        # ---- batched finals for this b ----
        r = fin.tile([128, NT], F32, tag="r")
        nc.vector.reciprocal_approx_fast(out=r[:], in_=dnc[:])
        cf = fin.tile([128, NT], F32, tag="cf")
        nc.gpsimd.tensor_mul(out=cf[:], in0=c1c[:], in1=r[:])
        cor = fin.tile([128, NT], F32, tag="cor")
        nc.gpsimd.tensor_mul(out=cor[:], in0=s1c[:], in1=r[:])
        dd = fin.tile([128, NT], F32, tag="dd")
        nc.gpsimd.tensor_sub(out=dd[:], in0=ivec_sb[:], in1=cor[:])
        dcl = fin.tile([128, NT], F32, tag="dcl")
        nc.gpsimd.tensor_scalar(
            out=dcl[:], in0=dd[:], scalar1=1.0 / W, scalar2=MIN_DISP,
            op0=mybir.AluOpType.mult, op1=mybir.AluOpType.max,
        )
        r2 = fin.tile([128, NT], F32, tag="r2")
        nc.vector.reciprocal_approx_fast(out=r2[:], in_=dcl[:])
        od = fin.tile([128, NT], F32, tag="od")
        nc.vector.tensor_scalar(
            out=od[:], in0=r2[:], scalar1=sc_sb[:, b:b + 1], scalar2=None,
            op0=mybir.AluOpType.mult,
        )
        nc.sync.dma_start(out=out_dc[b, :, :], in_=od[:])
        nc.sync.dma_start(out=out_cf[b, :, :], in_=cf[:])


_NC_CACHE = None


def _build_nc():
    global _NC_CACHE
    if _NC_CACHE is not None:
        return _NC_CACHE
    nc = bacc.Bacc("TRN2", target_bir_lowering=False, debug=False)
    io = {
        "lfeat": nc.dram_tensor("lfeat", (B, C, HLOC, W), F16, kind="ExternalInput"),
        "rfeat": nc.dram_tensor("rfeat", (B, C, HLOC, W), F16, kind="ExternalInput"),
        "widx": nc.dram_tensor("widx", (128, 4, W), F32, kind="ExternalInput"),
        "maskend": nc.dram_tensor("maskend", (128, 4), F32, kind="ExternalInput"),
        "ivec": nc.dram_tensor("ivec", (128, NT), F32, kind="ExternalInput"),
        "scales": nc.dram_tensor("scales", (1, B), F32, kind="ExternalInput"),
        "out_dc": nc.dram_tensor("out_dc", (B, 128, NT), F32, kind="ExternalOutput"),
        "out_cf": nc.dram_tensor("out_cf", (B, 128, NT), F32, kind="ExternalOutput"),
    }
    with tile.TileContext(nc) as tc:
        _body(tc, io)
    nc.compile()
    _NC_CACHE = nc
    return nc


def _host_constants():
    p = np.arange(128)[:, None, None]
    mi = np.arange(4)[None, :, None]
    j = np.arange(W)[None, None, :]
    widx = np.where(j <= mi * 128 + p, j, 0).astype(np.float32)       # (128,4,W)
    maskend = (np.arange(4)[None, :] * 128 + np.arange(128)[:, None] + 1).astype(
        np.float32)                                                    # (128,4)
    iv = (np.arange(4)[None, :] * 128 + np.arange(128)[:, None]).astype(np.float32)
    ivec = np.tile(iv, (1, HLOC))                                      # (128, 64)
    return widx, maskend, ivec


def kernel(feat, extri, intri, near, far, _run_kwargs=None, _core_ids=None):
    feat = np.asarray(feat, dtype=np.float32)
    extri = np.asarray(extri, dtype=np.float32)
    intri = np.asarray(intri, dtype=np.float32)
    far = np.asarray(far, dtype=np.float32)

    fx = intri[:, 0, 0, 0]                                             # (B,)
    baseline = np.linalg.norm(extri[:, 0, :3, 3] - extri[:, 1, :3, 3], axis=-1)
    lfar = far[:, 0]
    scales = (fx * baseline / lfar).astype(np.float32).reshape(1, B)

    widx, maskend, ivec = _host_constants()
    core_ids = list(range(NCORES)) if _core_ids is None else _core_ids

    feat_f16 = feat.astype(np.float16)                                 # (B,V,C,H,W)

    in_maps = []
    for ci in range(len(core_ids)):
        hs = slice(ci * HLOC, (ci + 1) * HLOC)
        in_maps.append({
            "lfeat": np.ascontiguousarray(feat_f16[:, 0, :, hs, :]),
            "rfeat": np.ascontiguousarray(feat_f16[:, 1, :, hs, :]),
            "widx": widx, "maskend": maskend, "ivec": ivec, "scales": scales,
        })

    nc = _build_nc()
    res = run_bass_kernel_spmd(nc, in_maps, core_ids=core_ids,
                               **(_run_kwargs or {}))

    out = np.zeros((B, 1, 2, H, W), dtype=np.float32)
    for ci in range(len(core_ids)):
        h0 = ci * HLOC
        dc = res.results[ci]["out_dc"]          # (B, 128, 64), col = h*4+mi
        cf = res.results[ci]["out_cf"]
        dc = dc.reshape(B, 128, HLOC, 4).transpose(0, 2, 3, 1).reshape(B, HLOC, W)
        cf = cf.reshape(B, 128, HLOC, 4).transpose(0, 2, 3, 1).reshape(B, HLOC, W)
        out[:, 0, 0, h0:h0 + HLOC, :] = dc
        out[:, 0, 1, h0:h0 + HLOC, :] = cf
    if _run_kwargs:
        kernel.last_results = res
    return out

